# Initial kernel scaffold
#
"""Chamfer loss kernel for Trainium2, 8 NeuronCores.

Math: for each batch b, P[i,j] = ||gt_i - pred_j||^2.
loss = mean_j min_i P[i,j] + mean_i min_j P[i,j]   (means over batch+point dims)

Device strategy (data parallel, 2 batches per core):
  M[i,j] = zz_ij - gg_i/2 - pp_j/2 = -P[i,j]/2 computed by ONE K=9 matmul:
    lhsT block "L" rows = (x, y, z, -0.5*ones x3, x^2, y^2, z^2-layout L)
    rhs  block "R" rows = (x, y, z, x^2, y^2, z^2, -0.5*ones x3)
  pairing: coords*coords + (-.5)*sq + sq*(-.5) = zz - gg/2 - pp/2.
  Both directions reuse the same two blocks with lhsT/rhs swapped
  (the pairing is symmetric). Row maxes of M = -rowmin(P)/2, reduced
  straight out of PSUM with DVE reduce_max, then summed per partition.
  Host: loss = -2 * sum(all partials) / (B*N).
"""

import sys

import numpy as np

sys.path.insert(0, "/opt/trn_rl_repo")

import concourse.bass as bass  # noqa: E402
import concourse.mybir as mybir  # noqa: E402
import concourse.tile as tile  # noqa: E402
from concourse import bacc  # noqa: E402
from concourse.bass_utils import run_bass_kernel_spmd  # noqa: E402

B, N_FULL, D = 16, 4096, 3
NCORES = 8
BLOC = B // NCORES  # batches per core
FREE = 512  # matmul moving free dim (one PSUM bank)
K = 9

_built = {}


def build(n=N_FULL, bloc=BLOC, group=2048):
    """Build the per-core Bass module. Returns (nc, names)."""
    key = (n, bloc, group)
    if key in _built:
        return _built[key]

    nt = n // 128  # row tiles per direction
    ngrp = n // group  # psum groups per row strip
    nch = group // FREE  # matmuls per psum group

    nc = bacc.Bacc("TRN2", target_bir_lowering=False, debug=False)
    dt = mybir.dt.float32

    gtsT = nc.dram_tensor("gtsT", [bloc, D, n], dt, kind="ExternalInput")
    predsT = nc.dram_tensor("predsT", [bloc, D, n], dt, kind="ExternalInput")
    out_dram = nc.dram_tensor("out", [128, 2 * bloc], dt, kind="ExternalOutput")

    with tile.TileContext(nc) as tc:
        with (
            tc.tile_pool(name="blocks", bufs=1) as blocks,
            tc.tile_pool(name="small", bufs=1) as small,
            tc.tile_pool(name="tmp", bufs=2) as tmp,
            tc.tile_pool(name="psum", bufs=2, space="PSUM") as psum_pool,
        ):
            out_sb = small.tile([128, 2 * bloc], dt, tag="out_sb")

            jobs = []  # (lhsT_block, rhs_block, out column)
            for b in range(bloc):
                gl = blocks.tile([K, n], dt, tag=f"gl{b}")
                pr = blocks.tile([K, n], dt, tag=f"pr{b}")
                # gtL rows: 0-2 coords, 3-5 const -0.5, 6-8 squares
                nc.sync.dma_start(out=gl[0:3, :], in_=gtsT[b])
                nc.gpsimd.memset(gl[3:6, :], -0.5)
                nc.gpsimd.tensor_tensor(
                    out=gl[6:9, :], in0=gl[0:3, :], in1=gl[0:3, :],
                    op=mybir.AluOpType.mult,
                )
                # predR rows: 0-2 coords, 3-5 squares, 6-8 const -0.5
                nc.sync.dma_start(out=pr[0:3, :], in_=predsT[b])
                nc.gpsimd.tensor_tensor(
                    out=pr[3:6, :], in0=pr[0:3, :], in1=pr[0:3, :],
                    op=mybir.AluOpType.mult,
                )
                nc.gpsimd.memset(pr[6:9, :], -0.5)
                jobs.append((gl, pr, 2 * b))  # dir A: rows = gts  -> loss_2
                jobs.append((pr, gl, 2 * b + 1))  # dir B: rows = preds -> loss_1

            for lhs_blk, rhs_blk, ocol in jobs:
                rowpart = tmp.tile([128, nt * ngrp], dt, tag="rowpart")
                for t in range(nt):
                    w = lhs_blk[:, t * 128:(t + 1) * 128]
                    for g in range(ngrp):
                        ps = psum_pool.tile([128, group], dt, tag="ps")
                        for c in range(nch):
                            j0 = (g * nch + c) * FREE
                            nc.tensor.matmul(
                                ps[:, c * FREE:(c + 1) * FREE],
                                w,
                                rhs_blk[:, j0:j0 + FREE],
                            )
                        col = t * ngrp + g
                        nc.vector.reduce_max(
                            rowpart[:, col:col + 1], ps[:],
                            axis=mybir.AxisListType.X,
                        )
                # combine: max over groups per tile, then sum over tiles
                if ngrp > 1:
                    rowmax = tmp.tile([128, nt], dt, tag="rowmax")
                    nc.vector.reduce_max(
                        rowmax[:],
                        rowpart[:].rearrange("p (t g) -> p t g", g=ngrp),
                        axis=mybir.AxisListType.X,
                    )
                else:
                    rowmax = rowpart
                nc.vector.reduce_sum(
                    out_sb[:, ocol:ocol + 1], rowmax[:],
                    axis=mybir.AxisListType.X,
                )

            nc.sync.dma_start(out=out_dram[:], in_=out_sb[:])

    nc.compile()
    _built[key] = (nc, gtsT.name, predsT.name, out_dram.name)
    return _built[key]


def shard_inputs(preds, gts, n=N_FULL, bloc=BLOC, ncores=NCORES):
    """Full [B, N, 3] inputs -> per-core in_maps with transposed layouts."""
    preds = np.ascontiguousarray(preds, dtype=np.float32)
    gts = np.ascontiguousarray(gts, dtype=np.float32)
    in_maps = []
    for c in range(ncores):
        lo = c * bloc
        pT = np.ascontiguousarray(preds[lo:lo + bloc].transpose(0, 2, 1))
        gT = np.ascontiguousarray(gts[lo:lo + bloc].transpose(0, 2, 1))
        in_maps.append({"predsT": pT, "gtsT": gT})
    return in_maps


def combine_outputs(outs, n=N_FULL, b=B):
    """outs: list of [128, 2*bloc] partial row-max sums -> scalar loss."""
    total = np.sum([o.astype(np.float64).sum() for o in outs])
    return np.float32(-2.0 * total / (b * n))


def kernel(preds, gts):
    nc, _, _, out_name = build()
    in_maps = shard_inputs(preds, gts)
    res = run_bass_kernel_spmd(nc, in_maps, core_ids=list(range(NCORES)))
    outs = [r[out_name] for r in res.results]
    return combine_outputs(outs)


if __name__ == "__main__":
    # quick numpy-vs-CoreSim self test on a small size
    from concourse.bass_interp import CoreSim

    n, bloc = 512, 1
    nc, gts_name, preds_name, out_name = build(n=n, bloc=bloc, group=512)
    rng = np.random.default_rng(0)
    preds = rng.standard_normal((bloc, n, D), dtype=np.float32)
    gts = rng.standard_normal((bloc, n, D), dtype=np.float32)

    sim = CoreSim(nc)
    sim.tensor(gts_name)[:] = gts.transpose(0, 2, 1)
    sim.tensor(preds_name)[:] = preds.transpose(0, 2, 1)
    sim.simulate()
    out = sim.tensor(out_name).copy()
    got = np.float32(-2.0 * out.astype(np.float64).sum() / (bloc * n))

    # numpy oracle
    def chamfer(preds, gts):
        tot = 0.0
        for b_ in range(preds.shape[0]):
            gg = (gts[b_] ** 2).sum(-1)
            pp = (preds[b_] ** 2).sum(-1)
            zz = gts[b_] @ preds[b_].T
            P = gg[:, None] + pp[None, :] - 2 * zz
            tot += P.min(axis=0).mean() + P.min(axis=1).mean()
        return tot / preds.shape[0]

    want = chamfer(preds, gts)
    print("sim:", got, "numpy:", want, "rel err:", abs(got - want) / abs(want))


# revision 8
# speedup vs baseline: 1.3269x; 1.3269x over previous
"""Chamfer loss kernel for Trainium2, 8 NeuronCores.

Math per batch: P[i,j] = ||a_i - b_j||^2, loss needs mean of row mins and
col mins. Device computes, for each direction, M[i,j] = a_i.b_j - ||b_j||^2/2
with ONE K=4 matmul per tile (lhsT rows = coords+ones, rhs rows = coords
+ (-|b|^2/2)). Then min_j P[i,:] = ||a_i||^2 - 2 max_j M[i,:], so
  sum_i min_j P = sum_i ||a_i||^2 - 2 * sum_i rowmax_i.
Row maxes are reduced straight out of PSUM with DVE reduce_max.
The -|b|^2/2 row is built on device: squares via gpsimd, summed over the
3 coords by a tiny K=3,M=1 matmul, scaled during the ACT PSUM->SBUF copy
(which also yields sum ||b||^2 for free via accum_out), then the 4-row rhs
block is assembled in a DRAM scratch tile (partition-offset writes are only
legal for DRAM/PSUM APs) and DMA'd back as one [4, n] SBUF block.

Sharding: batch dim 16 -> 2 per core. Host gathers per-core partial sums.
"""

import sys

import numpy as np

sys.path.insert(0, "/opt/trn_rl_repo")

import concourse.bass as bass  # noqa: E402
import concourse.mybir as mybir  # noqa: E402
import concourse.tile as tile  # noqa: E402
from concourse import bacc  # noqa: E402
from concourse.bass_utils import run_bass_kernel_spmd  # noqa: E402

B, N_FULL, D = 16, 4096, 3
NCORES = 8
BLOC = B // NCORES  # batches per core
FREE = 512  # matmul moving free dim (one PSUM bank)
K = 4

_built = {}


def build(n=N_FULL, bloc=BLOC, group=2048, variant="v2", reps=1):
    """Build the per-core Bass module. Returns (nc, in names, out names).

    variant "v2": rowmax via DVE reduce_max straight from PSUM (DVE-bound).
    variant "v3": most PSUM groups staged to SBUF by ScalarE copy, reduced
      by DVE tensor_scalar(op0=mult, op1=max, accum_out) which runs in
      2x_2P perf mode from SBUF; a fraction stays on the direct PSUM path
      so ACT and DVE finish together.
    reps: repeat the whole compute in a device-side For_i loop (timing aid).
    """
    key = (n, bloc, group, variant, reps)
    if key in _built:
        return _built[key]

    group = min(group, n)
    nt = n // 128  # row tiles per direction
    ngrp = n // group  # psum groups per row strip
    nch = group // FREE  # matmuls per psum group
    nmini = n // FREE  # chunks for the norm-row build

    nc = bacc.Bacc("TRN2", target_bir_lowering=False, debug=False)
    dt = mybir.dt.float32
    fp32 = mybir.dt.float32

    gtsE = nc.dram_tensor("gtsE", [bloc, K, n], dt, kind="ExternalInput")
    predsE = nc.dram_tensor("predsE", [bloc, K, n], dt, kind="ExternalInput")
    out_dram = nc.dram_tensor("out", [128, 2 * bloc], dt, kind="ExternalOutput")
    outs_dram = nc.dram_tensor("outs", [1, 2 * bloc], dt, kind="ExternalOutput")

    with tile.TileContext(nc) as tc:
        with (
            tc.tile_pool(name="blocks", bufs=1) as blocks,
            tc.tile_pool(name="prep", bufs=2) as prep,
            tc.tile_pool(name="small", bufs=1) as small,
            tc.tile_pool(name="tmp", bufs=2) as tmp,
            tc.tile_pool(name="psum", bufs=2, space="PSUM") as psum_pool,
            tc.tile_pool(name="dram", bufs=2, space="DRAM") as dram_pool,
        ):
            out_sb = small.tile([128, 2 * bloc], dt, tag="out_sb")
            out_scal = small.tile([1, 2 * bloc], dt, tag="out_scal")
            ones3 = small.tile([3, 1], dt, tag="ones3")
            nc.sync.dma_start(out=ones3[:], in_=gtsE[0, 3:4, 0:3].rearrange("a k -> k a"))

            jobs = []  # (lhsT block, rhs block, out col)
            for b in range(bloc):
                gl = blocks.tile([K, n], dt, tag=f"gl{b}")
                pl = blocks.tile([K, n], dt, tag=f"pl{b}")
                nc.sync.dma_start(out=gl[:], in_=gtsE[b])
                nc.sync.dma_start(out=pl[:], in_=predsE[b])

                # rhs blocks: coords + (-||x||^2/2) row, assembled via DRAM
                for li, (src, ext, rtag, scol) in enumerate(
                    [(pl, predsE, f"pr{b}", 2 * b + 1), (gl, gtsE, f"gr{b}", 2 * b)]
                ):
                    sq = prep.tile([3, n], fp32, tag="sq")
                    nc.gpsimd.tensor_tensor(
                        out=sq[:], in0=src[0:3, :], in1=src[0:3, :],
                        op=mybir.AluOpType.mult,
                    )
                    ppneg = prep.tile([1, n], fp32, tag="ppneg")
                    accrow = prep.tile([1, nmini], fp32, tag="accrow")
                    for c in range(nmini):
                        pps = psum_pool.tile([1, FREE], fp32, tag="ps")
                        nc.tensor.matmul(
                            pps[:], ones3[:], sq[:, c * FREE:(c + 1) * FREE]
                        )
                        nc.scalar.activation(
                            ppneg[:, c * FREE:(c + 1) * FREE], pps[:],
                            mybir.ActivationFunctionType.Copy, scale=-0.5,
                            accum_out=accrow[:, c:c + 1],
                        )
                    # sum ||x||^2 * (-1/2) for the lhs-side fixup term
                    nc.vector.reduce_sum(
                        out_scal[:, scol:scol + 1], accrow[:],
                        axis=mybir.AxisListType.X,
                    )
                    scr = dram_pool.tile([K, n], dt, tag="scr")
                    nc.sync.dma_start(out=scr[0:3, :], in_=ext[b, 0:3, :])
                    nc.sync.dma_start(out=scr[3:4, :], in_=ppneg[:])
                    rb = blocks.tile([K, n], dt, tag=rtag)
                    nc.sync.dma_start(out=rb[:], in_=scr[:])
                    if li == 0:
                        pr = rb
                    else:
                        gr = rb

                jobs.append((gl, pr, 2 * b))  # dir A: rows = gts  -> loss_2
                jobs.append((pl, gr, 2 * b + 1))  # dir B: rows = preds -> loss_1

            def emit_main():
                for lhs_blk, rhs_blk, ocol in jobs:
                    rowpart = tmp.tile([128, nt * ngrp], fp32, tag="rowpart")
                    for t in range(nt):
                        w = lhs_blk[:, t * 128:(t + 1) * 128]
                        for g in range(ngrp):
                            gi = t * ngrp + g
                            ps = psum_pool.tile([128, group], fp32, tag="ps")
                            for c in range(nch):
                                j0 = (g * nch + c) * FREE
                                nc.tensor.matmul(
                                    ps[:, c * FREE:(c + 1) * FREE],
                                    w,
                                    rhs_blk[:, j0:j0 + FREE],
                                )
                            direct = variant == "v2" or gi % 7 < 2
                            if direct:
                                nc.vector.reduce_max(
                                    rowpart[:, gi:gi + 1], ps[:],
                                    axis=mybir.AxisListType.X,
                                )
                            else:
                                stage = tmp.tile([128, group], fp32, tag="stage")
                                nc.scalar.copy(stage[:], ps[:])
                                scr = tmp.tile([128, group], fp32, tag="scratch")
                                nc.vector.tensor_scalar(
                                    out=scr[:], in0=stage[:], scalar1=1.0,
                                    scalar2=None, op0=mybir.AluOpType.mult,
                                    op1=mybir.AluOpType.max,
                                    accum_out=rowpart[:, gi:gi + 1],
                                )
                    # max over groups per tile, sum over tiles, then *-2
                    if ngrp > 1:
                        rowmax = tmp.tile([128, nt], fp32, tag="rowmax")
                        nc.vector.reduce_max(
                            rowmax[:],
                            rowpart[:].rearrange("p (t g) -> p t g", g=ngrp),
                            axis=mybir.AxisListType.X,
                        )
                    else:
                        rowmax = rowpart
                    s1 = tmp.tile([128, 1], fp32, tag="s1")
                    nc.vector.reduce_sum(
                        s1[:], rowmax[:], axis=mybir.AxisListType.X
                    )
                    nc.vector.tensor_scalar_mul(
                        out_sb[:, ocol:ocol + 1], s1[:], -2.0
                    )

            if reps == 1:
                emit_main()
            else:
                with tc.For_i(0, reps, 1):
                    emit_main()

            nc.sync.dma_start(out=out_dram[:], in_=out_sb[:])
            nc.sync.dma_start(out=outs_dram[:], in_=out_scal[:])

    nc.compile()
    _built[key] = (nc, "gtsE", "predsE", "out", "outs")
    return _built[key]


def make_ext(x):
    """[bloc, n, 3] coords -> [bloc, 4, n] block: rows 0-2 coordsT, row 3 ones."""
    bloc, n, _ = x.shape
    out = np.empty((bloc, K, n), dtype=np.float32)
    out[:, 0:3, :] = x.transpose(0, 2, 1)
    out[:, 3, :] = 1.0
    return np.ascontiguousarray(out)


def shard_inputs(preds, gts, bloc=BLOC, ncores=NCORES):
    preds = np.asarray(preds, dtype=np.float32)
    gts = np.asarray(gts, dtype=np.float32)
    in_maps = []
    for c in range(ncores):
        lo = c * bloc
        in_maps.append({
            "predsE": make_ext(preds[lo:lo + bloc]),
            "gtsE": make_ext(gts[lo:lo + bloc]),
        })
    return in_maps


def combine_outputs(outs, out_scals, n=N_FULL, b=B):
    t1 = np.sum([o.astype(np.float64).sum() for o in outs])
    t2 = np.sum([o.astype(np.float64).sum() for o in out_scals])
    return np.float32((t1 - 2.0 * t2) / (b * n))


def kernel(preds, gts):
    nc, _, _, o1, o2 = build()
    in_maps = shard_inputs(preds, gts)
    res = run_bass_kernel_spmd(nc, in_maps, core_ids=list(range(NCORES)))
    return combine_outputs(
        [r[o1] for r in res.results], [r[o2] for r in res.results]
    )


def _numpy_chamfer(preds, gts):
    tot = 0.0
    for b_ in range(preds.shape[0]):
        gg = (gts[b_] ** 2).sum(-1)
        pp = (preds[b_] ** 2).sum(-1)
        zz = gts[b_] @ preds[b_].T
        P = gg[:, None] + pp[None, :] - 2 * zz
        tot += P.min(axis=0).mean() + P.min(axis=1).mean()
    return tot / preds.shape[0]


if __name__ == "__main__":
    from concourse.bass_interp import CoreSim

    n = int(sys.argv[1]) if len(sys.argv) > 1 else 512
    bloc = int(sys.argv[2]) if len(sys.argv) > 2 else 1
    variant = sys.argv[3] if len(sys.argv) > 3 else "v2"
    reps = int(sys.argv[4]) if len(sys.argv) > 4 else 1
    nc, gn, pn, o1, o2 = build(n=n, bloc=bloc, variant=variant, reps=reps)
    rng = np.random.default_rng(0)
    preds = rng.standard_normal((bloc, n, D)).astype(np.float32)
    gts = rng.standard_normal((bloc, n, D)).astype(np.float32)

    sim = CoreSim(nc)
    sim.tensor(gn)[:] = make_ext(gts)
    sim.tensor(pn)[:] = make_ext(preds)
    sim.simulate()
    got = combine_outputs([sim.tensor(o1)], [sim.tensor(o2)], n=n, b=bloc)
    want = _numpy_chamfer(preds, gts)
    print("sim:", got, "numpy:", want, "rel err:", abs(got - want) / abs(want))
